# revision 1
# baseline (speedup 1.0000x reference)
"""Trainium2 Bass kernel for NNConv-style GNN message passing (8 NeuronCores).

Problem (from reference.py):
    N=10000 nodes, E=160000 edges, WIDTH=32, kernel-MLP 6->256->256->1024,
    DEPTH=4 message-passing iterations, scatter-mean aggregation.

Strategy (edge-parallel, dst-sorted):
  Host: sort edges by dst, shard contiguously so core k owns nodes
  [1280k, 1280k+1280) and all edges pointing into them; pad each 128-node
  window's edge list to a uniform (across cores) count so one SPMD program
  serves all 8 cores.

  Device, phase A (once): kernel MLP over edges -> per-edge 32x32 matrices
  stored fp16 in DRAM as W3T [(o,i), e] (o-major rows), computed with
  transposed activations so everything is natural PE matmuls.

  Device, per depth:
    - dma_gather source-node features from h4 [N, 128] (h replicated 4x
      along the row so one PE transpose of a gathered [128e,128] tile
      yields the [(rep,i), e] broadcast operand directly)
    - DVE multiply W3T-tile * hsrc-broadcast (fp16, 2x mode)
    - PE "mask matmul" reduces over i -> msgT [32, e] accumulated in PSUM
    - PE transpose msgT -> msg [e, 32]
    - DVE builds one-hot scatter matrices S^T[e, n] = (dst_local==n)/deg
      from an iota constant; PE matmul S^T.T @ msg accumulates the
      scatter-mean into a [128-node, 32] PSUM window; the root-weight term
      (h @ root_w + b) is one more matmul into the same PSUM group.
    - relu -> new h window -> AllGather h across the 8 cores.
  fc1/fc2 are folded in as tiny augmented matmuls (bias via ones-row).
"""

import sys, os

for _p in ("/opt/trn_rl_repo",):
    if _p not in sys.path and os.path.isdir(_p):
        sys.path.insert(0, _p)

import numpy as np

N = 10000
E = 160000
WIDTH = 32
KER_W = 256
KER_IN = 6
DEPTH = 4
N_CORES = 8
NPC = 1280           # nodes per core (8*1280 = 10240 >= 10000)
WIN = 128            # nodes per scatter window
NW = NPC // WIN      # windows per core


def _round_up(x, m):
    return ((x + m - 1) // m) * m


def host_prep(x, edge_index, edge_attr, fc1_w, fc1_b, k1_w, k1_b, k2_w, k2_b,
              k3_w, k3_b, root_w, conv_b, fc2_w, fc2_b,
              n=N, e=E, n_cores=N_CORES, npc=NPC):
    """Sort/shard/pad edges; build all per-core and constant arrays."""
    nw = npc // WIN
    n_pad = n_cores * npc

    src = np.asarray(edge_index[0], np.int64)
    dst = np.asarray(edge_index[1], np.int64)
    ea = np.asarray(edge_attr, np.float32)
    x = np.asarray(x, np.float32).reshape(-1)

    deg = np.bincount(dst, minlength=n).astype(np.float32)
    invdeg = (1.0 / np.maximum(deg, 1.0)).astype(np.float32)

    order = np.argsort(dst, kind="stable")
    dsts, srcs, eas = dst[order], src[order], ea[order]

    gw = dsts // WIN                      # global window id, 0 .. n_cores*nw-1
    counts = np.bincount(gw, minlength=n_cores * nw)
    # uniform-across-cores edges per window (SPMD: same trip counts)
    ew = [max(128, _round_up(int(counts[k * nw + w] if True else 0), 1))
          for k in range(n_cores) for w in range(nw)]
    EW = [max(128, _round_up(max(int(counts[k * nw + w]) for k in range(n_cores)), 128))
          for w in range(nw)]
    e_pc = sum(EW)
    ns_tot = e_pc // 128

    # window start offsets in the sorted arrays
    win_start = np.zeros(n_cores * nw + 1, np.int64)
    np.cumsum(counts, out=win_start[1:])

    # per-core padded arrays
    eaT_all, idx_all, dstl_all, invd_all, xw_all = [], [], [], [], []
    for k in range(n_cores):
        srcp = np.zeros(e_pc, np.int64)
        dstlp = np.zeros(e_pc, np.float32)
        invdp = np.zeros(e_pc, np.float32)
        eap = np.zeros((e_pc, KER_IN), np.float32)
        off = 0
        for w in range(nw):
            g = k * nw + w
            a, b = int(win_start[g]), int(win_start[g + 1])
            cnt = b - a
            srcp[off:off + cnt] = srcs[a:b]
            dstlp[off:off + cnt] = (dsts[a:b] - (k * npc + w * WIN)).astype(np.float32)
            invdp[off:off + cnt] = invdeg[dsts[a:b]]
            eap[off:off + cnt] = eas[a:b]
            off += EW[w]
        assert off == e_pc
        eaT_all.append(eap.T.astype(np.float16).copy())            # [6, e_pc]
        idx16 = srcp.astype(np.int16)                              # values < 10240
        idxw = idx16.reshape(e_pc // 16, 16).T.copy()              # [16, e_pc//16]
        idx_all.append(np.tile(idxw, (8, 1)).copy())               # [128, e_pc//16]
        dstl_all.append(dstlp.reshape(ns_tot, 128).T.copy())       # [128, ns_tot]
        invd_all.append(invdp.reshape(ns_tot, 128).T.copy())       # [128, ns_tot]
        xk = np.zeros((2, npc), np.float32)
        xs = x[k * npc: (k + 1) * npc]
        xk[0, :len(xs)] = xs
        xk[1, :] = 1.0
        xw_all.append(xk)

    # weights / constants (shared across cores)
    k3_perm = np.asarray(k3_w, np.float32).reshape(KER_W, WIDTH, WIDTH)  # [c, i, o]
    k3_perm = k3_perm.transpose(0, 2, 1).reshape(KER_W, WIDTH * WIDTH)   # cols (o,i)
    k3b_perm = np.asarray(k3_b, np.float32).reshape(WIDTH, WIDTH).T.reshape(-1)

    def wrap_pm(v, chunks):   # [chunks*128] -> [128, chunks] col-major per-partition
        return np.asarray(v, np.float32).reshape(chunks, 128).T.copy()

    def wrap_w(w_, chunks):   # [chunks*128, C] -> [128, chunks, C]
        w_ = np.asarray(w_, np.float32)
        return w_.reshape(chunks, 128, w_.shape[1]).transpose(1, 0, 2).astype(np.float16).copy()

    masks = np.zeros((128, 8 * 32), np.float16)
    for m in range(8):
        for p in range(128):
            masks[p, m * 32 + (4 * m + p // 32)] = 1.0
    consts = dict(
        k1w=np.asarray(k1_w, np.float16),                     # [6, 256]
        k1b=wrap_pm(k1_b, 2),                                 # [128, 2]
        k2w=wrap_w(k2_w, 2),                                  # [128, 2, 256]
        k2b=wrap_pm(k2_b, 2),
        k3w=wrap_w(k3_perm, 2),                               # [128, 2, 1024]
        k3b=wrap_pm(k3b_perm, 8),                             # [128, 8]
        masks=masks,
        iota=np.tile(np.arange(128, dtype=np.float32), (128, 1)),
        id128=np.eye(128, dtype=np.float16),
        id32=np.eye(32, dtype=np.float32),
        rootaug=np.vstack([np.asarray(root_w, np.float32),
                           np.asarray(conv_b, np.float32)[None, :]]),   # [33, 32]
        fc1aug=np.vstack([np.asarray(fc1_w, np.float32),
                          np.asarray(fc1_b, np.float32)[None, :]]),     # [2, 32]
        fc2aug=np.vstack([np.asarray(fc2_w, np.float32),
                          np.asarray(fc2_b, np.float32)[None, :]]),     # [33, 1]
    )

    cfg = dict(n_cores=n_cores, npc=npc, nw=nw, EW=EW, e_pc=e_pc,
               ns_tot=ns_tot, n_pad=n_pad)
    in_maps = []
    for k in range(n_cores):
        m = dict(consts)
        m.update(eaT=eaT_all[k], srcidx=idx_all[k], dstl=dstl_all[k],
                 invd=invd_all[k], xw=xw_all[k])
        in_maps.append(m)
    return cfg, in_maps


def build_program(cfg):
    import concourse.bass as bass
    import concourse.bacc as bacc
    import concourse.tile as tile
    import concourse.mybir as mybir
    from contextlib import ExitStack

    f16 = mybir.dt.float16
    f32 = mybir.dt.float32
    i16 = mybir.dt.int16
    AF = mybir.ActivationFunctionType
    OP = mybir.AluOpType

    n_cores, npc, nw = cfg["n_cores"], cfg["npc"], cfg["nw"]
    EW, e_pc, ns_tot = cfg["EW"], cfg["e_pc"], cfg["ns_tot"]
    n_pad = cfg["n_pad"]
    rg = [list(range(n_cores))]
    prof = cfg.get("profile_single", False)

    nc = bacc.Bacc("TRN2", target_bir_lowering=False, debug=False,
                   num_devices=1 if prof else n_cores)

    # --- I/O ---
    t_eaT = nc.dram_tensor("eaT", [KER_IN, e_pc], f16, kind="ExternalInput")
    t_idx = nc.dram_tensor("srcidx", [128, e_pc // 16], i16, kind="ExternalInput")
    t_dstl = nc.dram_tensor("dstl", [128, ns_tot], f32, kind="ExternalInput")
    t_invd = nc.dram_tensor("invd", [128, ns_tot], f32, kind="ExternalInput")
    t_k1w = nc.dram_tensor("k1w", [KER_IN, KER_W], f16, kind="ExternalInput")
    t_k1b = nc.dram_tensor("k1b", [128, 2], f32, kind="ExternalInput")
    t_k2w = nc.dram_tensor("k2w", [128, 2, KER_W], f16, kind="ExternalInput")
    t_k2b = nc.dram_tensor("k2b", [128, 2], f32, kind="ExternalInput")
    t_k3w = nc.dram_tensor("k3w", [128, 2, 1024], f16, kind="ExternalInput")
    t_k3b = nc.dram_tensor("k3b", [128, 8], f32, kind="ExternalInput")
    t_masks = nc.dram_tensor("masks", [128, 256], f16, kind="ExternalInput")
    t_iota = nc.dram_tensor("iota", [128, 128], f32, kind="ExternalInput")
    t_id128 = nc.dram_tensor("id128", [128, 128], f16, kind="ExternalInput")
    t_id32 = nc.dram_tensor("id32", [32, 32], f32, kind="ExternalInput")
    t_raug = nc.dram_tensor("rootaug", [33, 32], f32, kind="ExternalInput")
    t_f1 = nc.dram_tensor("fc1aug", [2, 32], f32, kind="ExternalInput")
    t_f2 = nc.dram_tensor("fc2aug", [33, 1], f32, kind="ExternalInput")
    t_xw = nc.dram_tensor("xw", [2, npc], f32, kind="ExternalInput")
    t_y = nc.dram_tensor("y", [npc, 1], f32, kind="ExternalOutput")

    ecum = np.zeros(nw + 1, np.int64)
    np.cumsum(EW, out=ecum[1:])

    with tile.TileContext(nc) as tc, ExitStack() as ctx:
        sb = ctx.enter_context(tc.tile_pool(name="sb", bufs=2))
        cb = ctx.enter_context(tc.tile_pool(name="cb", bufs=1))   # constants
        ps = ctx.enter_context(tc.tile_pool(name="ps", bufs=2,
                                            space=bass.MemorySpace.PSUM))
        dr = ctx.enter_context(tc.tile_pool(name="dr", bufs=1,
                                            space=bass.MemorySpace.DRAM))

        # ---- internal DRAM ----
        w3_dram = dr.tile([1024, e_pc], f16, name="w3_dram")
        w3v = w3_dram.rearrange("(c p) e -> p c e", p=128)
        h4own = [dr.tile([npc, 128], f16, name=f"h4own{d}", tag=f"h4own{d}")
                 for d in range(DEPTH + 1)]
        h4full = [dr.tile([n_pad, 128], f16, name=f"h4full{d}",
                          addr_space="Shared", tag=f"h4full{d}")
                  for d in range(DEPTH)]

        # ---- resident constants ----
        def load_const(t, shape, dtype, name):
            s = cb.tile(shape, dtype, name=name)
            nc.sync.dma_start(s[:], t.ap())
            return s

        k1w_s = load_const(t_k1w, [KER_IN, KER_W], f16, "k1w_s")
        k1b_s = load_const(t_k1b, [128, 2], f32, "k1b_s")
        k2w_s = load_const(t_k2w, [128, 2, KER_W], f16, "k2w_s")
        k2b_s = load_const(t_k2b, [128, 2], f32, "k2b_s")
        k3w_s = load_const(t_k3w, [128, 2, 1024], f16, "k3w_s")
        k3b_s = load_const(t_k3b, [128, 8], f32, "k3b_s")
        masks_s = load_const(t_masks, [128, 256], f16, "masks_s")
        iota_s = load_const(t_iota, [128, 128], f32, "iota_s")
        id128_s = load_const(t_id128, [128, 128], f16, "id128_s")
        id32_s = load_const(t_id32, [32, 32], f32, "id32_s")
        raug_s = load_const(t_raug, [33, 32], f32, "raug_s")
        f1_s = load_const(t_f1, [2, 32], f32, "f1_s")
        f2_s = load_const(t_f2, [33, 1], f32, "f2_s")
        xw_s = load_const(t_xw, [2, npc], f32, "xw_s")
        idx_s = load_const(t_idx, [128, e_pc // 16], i16, "idx_s")
        dstl_s = load_const(t_dstl, [128, ns_tot], f32, "dstl_s")
        invd_s = load_const(t_invd, [128, ns_tot], f32, "invd_s")

        # ================= phase A: kernel MLP -> W3T in DRAM =================
        for e0 in range(0, e_pc, 512):
            nt = min(512, e_pc - e0)
            ea_t = sb.tile([KER_IN, nt], f16, tag="ea", name="ea_t")
            nc.sync.dma_start(ea_t[:], t_eaT.ap()[:, e0:e0 + nt])

            h1_t = sb.tile([128, 2, nt], f16, tag="h1", name="h1_t")
            for mo in range(2):
                p1 = ps.tile([128, nt], f32, tag="pbig", name="p1")
                nc.tensor.matmul(p1[:], k1w_s[:, mo * 128:(mo + 1) * 128],
                                 ea_t[:], start=True, stop=True)
                nc.scalar.activation(h1_t[:, mo, :], p1[:], AF.Relu,
                                     bias=k1b_s[:, mo:mo + 1])
            h2_t = sb.tile([128, 2, nt], f16, tag="h2", name="h2_t")
            for mo in range(2):
                p2 = ps.tile([128, nt], f32, tag="pbig", name="p2")
                for mi in range(2):
                    nc.tensor.matmul(p2[:], k2w_s[:, mi, mo * 128:(mo + 1) * 128],
                                     h1_t[:, mi, :], start=(mi == 0), stop=(mi == 1))
                nc.scalar.activation(h2_t[:, mo, :], p2[:], AF.Relu,
                                     bias=k2b_s[:, mo:mo + 1])
            for mo in range(8):
                p3 = ps.tile([128, nt], f32, tag="pbig", name="p3")
                for mi in range(2):
                    nc.tensor.matmul(p3[:], k3w_s[:, mi, mo * 128:(mo + 1) * 128],
                                     h2_t[:, mi, :], start=(mi == 0), stop=(mi == 1))
                w3o = sb.tile([128, nt], f16, tag="w3o", name="w3o")
                nc.scalar.activation(w3o[:], p3[:], AF.Identity,
                                     bias=k3b_s[:, mo:mo + 1])
                nc.sync.dma_start(w3v[:, mo, e0:e0 + nt], w3o[:])

        # ================= init: h0 = x @ fc1 + b =================
        for w in range(nw):
            p0 = ps.tile([128, 32], f32, tag="pwin", name="p0")
            nc.tensor.matmul(p0[:], xw_s[:, w * 128:(w + 1) * 128], f1_s[:],
                             start=True, stop=True)
            h0 = sb.tile([128, 128], f16, tag="hnew", name="h0")
            nc.scalar.copy(h0[:, 0:32], p0[:])
            for r in range(1, 4):
                nc.vector.tensor_copy(h0[:, 32 * r:32 * (r + 1)], h0[:, 0:32])
            nc.sync.dma_start(h4own[0][w * 128:(w + 1) * 128, :], h0[:])
        if not prof:
            nc.gpsimd.collective_compute(
                "AllGather", mybir.AluOpType.bypass, replica_groups=rg,
                ins=[h4own[0].opt()], outs=[h4full[0].opt()])

        # ================= message-passing depths =================
        for d in range(DEPTH):
            hsrc_dram = h4full[d]
            for w in range(nw):
                n_sub = EW[w] // 128
                pwin = ps.tile([128, 32], f32, tag="pwin", name="pwin")
                first = True
                for t0 in range(0, n_sub, 4):
                    nst = min(4, n_sub - t0)
                    ntv = nst * 128
                    e0 = int(ecum[w]) + t0 * 128
                    # loads
                    w3t = sb.tile([128, 8, ntv], f16, tag="w3t", name="w3t")
                    nc.sync.dma_start(w3t[:], w3v[:, :, e0:e0 + ntv])
                    g_t = sb.tile([128, 1, ntv], f16, tag="g", name="g_t")
                    nc.gpsimd.dma_gather(
                        g_t[:], hsrc_dram[:, :],
                        idx_s[:, e0 // 16:(e0 + ntv) // 16],
                        num_idxs=ntv, num_idxs_reg=ntv, elem_size=128,
                        transpose=True)
                    # xbar-transposed gather: g_t[:, 0, :] is already the
                    # [(rep,i), e] broadcast operand
                    tmp = sb.tile([128, 8, ntv], f16, tag="tmp", name="tmp")
                    for m in range(8):
                        nc.vector.tensor_tensor(tmp[:, m, :], w3t[:, m, :],
                                                g_t[:, 0, :], mybir.AluOpType.mult)
                    # msgT = sum_i tmp  (PE mask matmuls)
                    pmsgT = ps.tile([32, ntv], f32, tag="pbig", name="pmsgT")
                    for m in range(8):
                        nc.tensor.matmul(pmsgT[:], masks_s[:, m * 32:(m + 1) * 32],
                                         tmp[:, m, :], start=(m == 0), stop=(m == 7))
                    msgT = sb.tile([32, ntv], f32, tag="msgT", name="msgT")
                    nc.scalar.copy(msgT[:], pmsgT[:])
                    # per-subtile: transpose msg, build S^T, scatter-accumulate
                    for s in range(nst):
                        gs = e0 // 128 + s
                        pmsg = ps.tile([128, 32], f32, tag="pmsg", name="pmsg")
                        nc.tensor.transpose(pmsg[:], msgT[:, s * 128:(s + 1) * 128],
                                            id32_s[:])
                        msg = sb.tile([128, 32], f32, tag="msg", name="msg")
                        nc.scalar.copy(msg[:], pmsg[:])
                        st = sb.tile([128, 128], f32, tag="st", name="st")
                        nc.vector.tensor_scalar(
                            st[:], iota_s[:], dstl_s[:, gs:gs + 1],
                            invd_s[:, gs:gs + 1], op0=OP.is_equal, op1=OP.mult)
                        nc.tensor.matmul(pwin[:], st[:], msg[:],
                                         start=first, stop=False)
                        first = False
                # window tail: + h @ root_w + b, relu, store
                hw_t = sb.tile([128, 32], f16, tag="hw", name="hw_t")
                nc.sync.dma_start(
                    hw_t[:], h4own[d][w * 128:(w + 1) * 128, 0:32])
                pth = ps.tile([32, 128], f16, tag="ptp", name="pth")
                nc.tensor.transpose(pth[:], hw_t[:], id128_s[:])
                htaug = sb.tile([33, 128], f32, tag="htaug", name="htaug")
                nc.scalar.copy(htaug[0:32, :], pth[:])
                nc.gpsimd.memset(htaug[32:33, :], 1.0)
                nc.tensor.matmul(pwin[:], htaug[:], raug_s[:],
                                 start=False, stop=True)
                hnew = sb.tile([128, 128], f16, tag="hnew", name="hnew")
                nc.scalar.activation(hnew[:, 0:32], pwin[:], AF.Relu)
                if d < DEPTH - 1:
                    for r in range(1, 4):
                        nc.vector.tensor_copy(hnew[:, 32 * r:32 * (r + 1)],
                                              hnew[:, 0:32])
                    nc.sync.dma_start(
                        h4own[d + 1][w * 128:(w + 1) * 128, :], hnew[:])
                else:
                    # final depth: fuse fc2
                    pty = ps.tile([32, 128], f16, tag="ptp", name="pty")
                    nc.tensor.transpose(pty[:], hnew[:, 0:32], id128_s[:])
                    htaug2 = sb.tile([33, 128], f32, tag="htaug", name="htaug2")
                    nc.scalar.copy(htaug2[0:32, :], pty[:])
                    nc.gpsimd.memset(htaug2[32:33, :], 1.0)
                    py = ps.tile([128, 1], f32, tag="pmsg", name="py")
                    nc.tensor.matmul(py[:], htaug2[:], f2_s[:],
                                     start=True, stop=True)
                    y_sb = sb.tile([128, 1], f32, tag="ysb", name="y_sb")
                    nc.scalar.copy(y_sb[:], py[:])
                    nc.sync.dma_start(t_y.ap()[w * 128:(w + 1) * 128, :], y_sb[:])
            if d < DEPTH - 1 and not prof:
                nc.gpsimd.collective_compute(
                    "AllGather", mybir.AluOpType.bypass, replica_groups=rg,
                    ins=[h4own[d + 1].opt()], outs=[h4full[d + 1].opt()])

    nc.compile()
    return nc


_CACHE = {}


def _get_program(cfg):
    key = (cfg["e_pc"], tuple(cfg["EW"]), cfg["n_cores"], cfg["npc"])
    if key not in _CACHE:
        _CACHE[key] = build_program(cfg)
    return _CACHE[key]


def kernel(**inputs):
    from concourse import bass_utils
    cfg, in_maps = host_prep(**inputs)
    nc = _get_program(cfg)
    res = bass_utils.run_bass_kernel_spmd(
        nc, in_maps, core_ids=list(range(cfg["n_cores"])))
    npc, n_cores = cfg["npc"], cfg["n_cores"]
    y = np.zeros((N, 1), np.float32)
    for k in range(n_cores):
        lo = k * npc
        hi = min(lo + npc, N)
        if hi > lo:
            y[lo:hi, 0] = res.results[k]["y"][:hi - lo, 0]
    return y



# revision 12
# speedup vs baseline: 853.7897x; 853.7897x over previous
"""Trainium2 Bass kernel for NNConv-style GNN message passing (8 NeuronCores).

Problem (from the reference):
    N=10000 nodes, E=160000 edges, WIDTH=32, kernel-MLP 6->256->256->1024,
    DEPTH=4 message-passing iterations, scatter-mean aggregation.

Strategy (edge-parallel, dst-sorted):
  Host: sort edges by dst, shard contiguously so core k owns nodes
  [1280k, 1280k+1280) and all edges pointing into them; pad each 128-node
  window's edge list to a uniform (across cores) count so one SPMD program
  serves all 8 cores.

  Device, per core:
   - init: h0 = x @ fc1 + b per window; AllGather h across cores (h rows
     are replicated 4x to width 128 so a transposed dma_gather of a
     [128, e] tile directly yields the [(rep,i), e] broadcast operand).
   - scatter matrices S^T[e, n] = (dst_local==n)*invdeg (one [128,128]
     tile per 128-edge subtile) are built once by DVE from an iota
     constant and kept resident in SBUF for all 4 depths.
   - phase A fused with depth 0, per 128-node window: kernel-MLP over the
     window's edges -> per-edge 32x32 matrices W3T [(o,i), e] held in
     SBUF [128, 8, EW] (o-major row chunks), written once to DRAM
     (contiguous per partition -> cheap descriptors) for depths 1-3, and
     consumed immediately by the depth-0 message pass.
   - message pass, per window: dma_gather source features; DVE multiplies
     W3T-chunks by the gathered broadcast operand; PE mask-matmuls reduce
     over i into msgT [32, e]; PE transposes (batched into one PSUM tile)
     yield msg [e, 32]; S^T matmuls accumulate the scatter-mean into a
     [128, 32] PSUM window; the root-weight term (h @ root_w + b) is one
     more matmul into the same PSUM group (hT kept resident in SBUF with
     an augmented ones row); relu -> new h window -> AllGather.
  fc1/fc2 are folded in as tiny augmented matmuls.

Run path: the axon PJRT redirect of run_bass_kernel_spmd rebuilds the jit
wrapper per call (~1s overhead). kernel() instead builds the same
shard_map'd bass_exec jit once and caches it (same lowering path,
hoisted out of the loop). `reps` builds a program that executes the
whole kernel R times back-to-back, used by the harness to measure the
marginal (steady-state) per-execution device time.
"""

import sys, os

for _p in ("/opt/trn_rl_repo",):
    if _p not in sys.path and os.path.isdir(_p):
        sys.path.insert(0, _p)

import numpy as np

N = 10000
E = 160000
WIDTH = 32
KER_W = 256
KER_IN = 6
DEPTH = 4
N_CORES = 8
NPC = 1280           # nodes per core (8*1280 = 10240 >= 10000)
WIN = 128            # nodes per scatter window
NW = NPC // WIN      # windows per core
W3_SCALE = 8.0       # fp8(e3m4) W3 storage scale (undone exactly by masks=1/8)


def _round_up(x, m):
    return ((x + m - 1) // m) * m


def host_prep(x, edge_index, edge_attr, fc1_w, fc1_b, k1_w, k1_b, k2_w, k2_b,
              k3_w, k3_b, root_w, conv_b, fc2_w, fc2_b,
              n=N, e=E, n_cores=N_CORES, npc=NPC):
    """Sort/shard/pad edges; build all per-core and constant arrays."""
    nw = npc // WIN
    n_pad = n_cores * npc

    src = np.asarray(edge_index[0], np.int64)
    dst = np.asarray(edge_index[1], np.int64)
    ea = np.asarray(edge_attr, np.float32)
    x = np.asarray(x, np.float32).reshape(-1)

    deg = np.bincount(dst, minlength=n).astype(np.float32)
    invdeg = (1.0 / np.maximum(deg, 1.0)).astype(np.float32)

    order = np.argsort(dst, kind="stable")
    dsts, srcs, eas = dst[order], src[order], ea[order]

    gw = dsts // WIN                      # global window id, 0 .. n_cores*nw-1
    counts = np.bincount(gw, minlength=n_cores * nw)
    # uniform-across-cores edges per window (SPMD: same trip counts)
    EW = [max(128, _round_up(max(int(counts[k * nw + w]) for k in range(n_cores)), 128))
          for w in range(nw)]
    e_pc = sum(EW)
    ns_tot = e_pc // 128

    # window start offsets in the sorted arrays
    win_start = np.zeros(n_cores * nw + 1, np.int64)
    np.cumsum(counts, out=win_start[1:])

    # per-core padded arrays
    eaT_all, idx_all, dstl_all, invd_all, xw_all = [], [], [], [], []
    for k in range(n_cores):
        srcp = np.zeros(e_pc, np.int64)
        dstlp = np.zeros(e_pc, np.float32)
        invdp = np.zeros(e_pc, np.float32)
        eap = np.zeros((e_pc, KER_IN), np.float32)
        off = 0
        for w in range(nw):
            g = k * nw + w
            a, b = int(win_start[g]), int(win_start[g + 1])
            cnt = b - a
            srcp[off:off + cnt] = srcs[a:b]
            dstlp[off:off + cnt] = (dsts[a:b] - (k * npc + w * WIN)).astype(np.float32)
            invdp[off:off + cnt] = invdeg[dsts[a:b]]
            eap[off:off + cnt] = eas[a:b]
            off += EW[w]
        assert off == e_pc
        eaT_all.append(eap.T.astype(np.float16).copy())            # [6, e_pc]
        idx16 = srcp.astype(np.int16)                              # values < 10240
        idxw = idx16.reshape(e_pc // 16, 16).T.copy()              # [16, e_pc//16]
        idx_all.append(np.tile(idxw, (8, 1)).copy())               # [128, e_pc//16]
        dstl_all.append(dstlp.reshape(ns_tot, 128).T.copy())       # [128, ns_tot]
        invd_all.append(invdp.reshape(ns_tot, 128).T.copy())       # [128, ns_tot]
        xk = np.zeros((2, npc), np.float32)
        xs = x[k * npc: (k + 1) * npc]
        xk[0, :len(xs)] = xs
        xk[1, :] = 1.0
        xw_all.append(xk)

    # weights / constants (shared across cores)
    k3_perm = np.asarray(k3_w, np.float32).reshape(KER_W, WIDTH, WIDTH)  # [c, i, o]
    k3_perm = k3_perm.transpose(0, 2, 1).reshape(KER_W, WIDTH * WIDTH)   # cols (o,i)
    k3b_perm = np.asarray(k3_b, np.float32).reshape(WIDTH, WIDTH).T.reshape(-1)

    def wrap_pm(v, chunks):   # [chunks*128] -> [128, chunks] col-major per-partition
        return np.asarray(v, np.float32).reshape(chunks, 128).T.copy()

    def wrap_w(w_, chunks):   # [chunks*128, C] -> [128, chunks, C]
        w_ = np.asarray(w_, np.float32)
        return w_.reshape(chunks, 128, w_.shape[1]).transpose(1, 0, 2).astype(np.float16).copy()

    masks = np.zeros((128, 8 * 32), np.float16)
    for m in range(8):
        for p in range(128):
            masks[p, m * 32 + (4 * m + p // 32)] = 1.0 / W3_SCALE
    consts = dict(
        k1w=np.asarray(k1_w, np.float16),                     # [6, 256]
        k1b=wrap_pm(k1_b, 2),                                 # [128, 2]
        k2w=wrap_w(k2_w, 2),                                  # [128, 2, 256]
        k2b=wrap_pm(k2_b, 2),
        k3w=wrap_w(k3_perm, 2),                               # [128, 2, 1024]
        k3b=wrap_pm(k3b_perm * W3_SCALE, 8),                  # [128, 8] (prescaled)
        masks=masks,
        iota=np.tile(np.arange(128, dtype=np.float32), (128, 1)),
        id128=np.eye(128, dtype=np.float16),
        id32=np.eye(32, dtype=np.float32),
        rootaug=np.vstack([np.asarray(root_w, np.float32),
                           np.asarray(conv_b, np.float32)[None, :]]),   # [33, 32]
        fc1aug=np.vstack([np.asarray(fc1_w, np.float32),
                          np.asarray(fc1_b, np.float32)[None, :]]),     # [2, 32]
        fc2aug=np.vstack([np.asarray(fc2_w, np.float32),
                          np.asarray(fc2_b, np.float32)[None, :]]),     # [33, 1]
    )

    cfg = dict(n_cores=n_cores, npc=npc, nw=nw, EW=EW, e_pc=e_pc,
               ns_tot=ns_tot, n_pad=n_pad)
    in_maps = []
    for k in range(n_cores):
        m = dict(consts)
        m.update(eaT=eaT_all[k], srcidx=idx_all[k], dstl=dstl_all[k],
                 invd=invd_all[k], xw=xw_all[k])
        in_maps.append(m)
    return cfg, in_maps


def build_program(cfg, reps=1):
    import concourse.bass as bass
    import concourse.bacc as bacc
    import concourse.tile as tile
    import concourse.mybir as mybir
    from contextlib import ExitStack

    f16 = mybir.dt.float16
    f32 = mybir.dt.float32
    f8 = mybir.dt.float8e3
    i16 = mybir.dt.int16
    AF = mybir.ActivationFunctionType
    OP = mybir.AluOpType

    n_cores, npc, nw = cfg["n_cores"], cfg["npc"], cfg["nw"]
    EW, e_pc, ns_tot = cfg["EW"], cfg["e_pc"], cfg["ns_tot"]
    n_pad = cfg["n_pad"]
    rg = [list(range(n_cores))]
    prof = cfg.get("profile_single", False)
    w3_fp8 = cfg.get("w3_fp8", True)
    st_resident = cfg.get("st_resident", True)
    hT_resident = cfg.get("hT_resident", True)

    nc = bacc.Bacc("TRN2", target_bir_lowering=False, debug=False,
                   num_devices=1 if prof else n_cores)

    # --- I/O ---
    t_eaT = nc.dram_tensor("eaT", [KER_IN, e_pc], f16, kind="ExternalInput")
    t_idx = nc.dram_tensor("srcidx", [128, e_pc // 16], i16, kind="ExternalInput")
    t_dstl = nc.dram_tensor("dstl", [128, ns_tot], f32, kind="ExternalInput")
    t_invd = nc.dram_tensor("invd", [128, ns_tot], f32, kind="ExternalInput")
    t_k1w = nc.dram_tensor("k1w", [KER_IN, KER_W], f16, kind="ExternalInput")
    t_k1b = nc.dram_tensor("k1b", [128, 2], f32, kind="ExternalInput")
    t_k2w = nc.dram_tensor("k2w", [128, 2, KER_W], f16, kind="ExternalInput")
    t_k2b = nc.dram_tensor("k2b", [128, 2], f32, kind="ExternalInput")
    t_k3w = nc.dram_tensor("k3w", [128, 2, 1024], f16, kind="ExternalInput")
    t_k3b = nc.dram_tensor("k3b", [128, 8], f32, kind="ExternalInput")
    t_masks = nc.dram_tensor("masks", [128, 256], f16, kind="ExternalInput")
    t_iota = nc.dram_tensor("iota", [128, 128], f32, kind="ExternalInput")
    t_id128 = nc.dram_tensor("id128", [128, 128], f16, kind="ExternalInput")
    t_id32 = nc.dram_tensor("id32", [32, 32], f32, kind="ExternalInput")
    t_raug = nc.dram_tensor("rootaug", [33, 32], f32, kind="ExternalInput")
    t_f1 = nc.dram_tensor("fc1aug", [2, 32], f32, kind="ExternalInput")
    t_f2 = nc.dram_tensor("fc2aug", [33, 1], f32, kind="ExternalInput")
    t_xw = nc.dram_tensor("xw", [2, npc], f32, kind="ExternalInput")
    t_y = nc.dram_tensor("y", [npc, 1], f32, kind="ExternalOutput")

    ecum = np.zeros(nw + 1, np.int64)
    np.cumsum(EW, out=ecum[1:])

    with tile.TileContext(nc) as tc, ExitStack() as ctx:
        sb = ctx.enter_context(tc.tile_pool(name="sb", bufs=2))
        cb = ctx.enter_context(tc.tile_pool(name="cb", bufs=1))   # constants
        ps = ctx.enter_context(tc.tile_pool(name="ps", bufs=2,
                                            space=bass.MemorySpace.PSUM))
        dr = ctx.enter_context(tc.tile_pool(name="dr", bufs=1,
                                            space=bass.MemorySpace.DRAM))

        # ---- internal DRAM ----
        # per-window W3T buffers, [128, 8, EW[w]]: contiguous per partition
        w3w_dram = [dr.tile([128, 8, EW[w]], f8 if w3_fp8 else f16,
                    name=f"w3w{w}", tag=f"w3w{w}")
                    for w in range(nw)]
        h4own_r = [[dr.tile([npc, 128], f16, name=f"h4own{d}_r{r}",
                                    tag=f"h4own{d}_r{r}") for d in range(DEPTH)]
                   for r in range(reps)]
        h4full_r = [[dr.tile([n_pad, 128], f16, name=f"h4full{d}_r{r}",
                             addr_space="Shared", tag=f"h4full{d}_r{r}")
                     for d in range(DEPTH)]
                    for r in range(reps)]

        # ---- resident constants ----
        def load_const(t, shape, dtype, name):
            s = cb.tile(shape, dtype, name=name)
            nc.sync.dma_start(s[:], t.ap())
            return s

        k1w_s = load_const(t_k1w, [KER_IN, KER_W], f16, "k1w_s")
        k1b_s = load_const(t_k1b, [128, 2], f32, "k1b_s")
        k2w_s = load_const(t_k2w, [128, 2, KER_W], f16, "k2w_s")
        k2b_s = load_const(t_k2b, [128, 2], f32, "k2b_s")
        k3w_s = load_const(t_k3w, [128, 2, 1024], f16, "k3w_s")
        k3b_s = load_const(t_k3b, [128, 8], f32, "k3b_s")
        masks_s = load_const(t_masks, [128, 256], f16, "masks_s")
        iota_s = load_const(t_iota, [128, 128], f32, "iota_s")
        id128_s = load_const(t_id128, [128, 128], f16, "id128_s")
        id32_s = load_const(t_id32, [32, 32], f32, "id32_s")
        raug_s = load_const(t_raug, [33, 32], f32, "raug_s")
        f1_s = load_const(t_f1, [2, 32], f32, "f1_s")
        f2_s = load_const(t_f2, [33, 1], f32, "f2_s")
        xw_s = load_const(t_xw, [2, npc], f32, "xw_s")
        idx_s = load_const(t_idx, [128, e_pc // 16], i16, "idx_s")
        dstl_s = load_const(t_dstl, [128, ns_tot], f32, "dstl_s")
        invd_s = load_const(t_invd, [128, ns_tot], f32, "invd_s")

        # resident scatter matrices (built once; reused all depths/reps):
        # st_res[:, gs, :] = (dstl[e]==n) * invdeg[dst[e]]  for subtile gs
        if st_resident:
            st_res = cb.tile([128, ns_tot, 128], f16, name="st_res")
            for gs in range(ns_tot):
                nc.vector.tensor_scalar(
                    st_res[:, gs, :], iota_s[:], dstl_s[:, gs:gs + 1],
                    invd_s[:, gs:gs + 1], op0=OP.is_equal, op1=OP.mult)

        # resident transposed h (augmented with a ones row for biases)
        if hT_resident:
            hT_res = cb.tile([33, npc], f32, name="hT_res")
            nc.gpsimd.memset(hT_res[32:33, :], 1.0)

        # inner tiles per window: [(e0, nt), ...] with nt <= 512
        def inner_tiles(w):
            out, e0 = [], 0
            while e0 < EW[w]:
                nt = min(512, EW[w] - e0)
                out.append((e0, nt))
                e0 += nt
            return out

        for rep in range(reps):
            h4own = h4own_r[rep]
            h4full = h4full_r[rep]
            # ================= init: h0 = x @ fc1 + b =================
            for w in range(nw):
                p0 = ps.tile([128, 32], f32, tag="pwin", name="p0")
                nc.tensor.matmul(p0[:], xw_s[:, w * 128:(w + 1) * 128], f1_s[:],
                                 start=True, stop=True)
                h0 = sb.tile([128, 128], f16, tag="hnew", name="h0")
                nc.scalar.copy(h0[:, 0:32], p0[:])
                for r in range(1, 4):
                    nc.vector.tensor_copy(h0[:, 32 * r:32 * (r + 1)], h0[:, 0:32])
                nc.sync.dma_start(h4own[0][w * 128:(w + 1) * 128, :], h0[:])
                if hT_resident:
                    # hT_res update: transpose h0 -> [32, 128]
                    pth = ps.tile([32, 128], f16, tag="ptp", name="pth")
                    nc.tensor.transpose(pth[:], h0[:, 0:32], id128_s[:])
                    nc.scalar.copy(hT_res[0:32, w * 128:(w + 1) * 128], pth[:])
            if not prof:
                nc.gpsimd.collective_compute(
                    "AllGather", mybir.AluOpType.bypass, replica_groups=rg,
                    ins=[h4own[0].opt()], outs=[h4full[0].opt()])

            # ============ message-passing depth body ============
            def depth_window(d, w, w3win):
                """Message pass for window w at depth d; w3win is the SBUF
                [128, 8, EW[w]] per-edge weight tile."""
                e_base = int(ecum[w])
                g_t = sb.tile([128, 1, EW[w]], f16, tag="g", name="g_t")
                for g0 in range(0, EW[w], 512):
                    gn = min(512, EW[w] - g0)
                    nc.gpsimd.dma_gather(
                        g_t[:, :, g0:g0 + gn], h4full[d][:, :],
                        idx_s[:, (e_base + g0) // 16:(e_base + g0 + gn) // 16],
                        num_idxs=gn, num_idxs_reg=gn, elem_size=128,
                        transpose=True)
                pwin = ps.tile([128, 32], f32, tag="pwin", name="pwin")
                first = True
                for (e0, nt) in inner_tiles(w):
                    nst = (nt + 127) // 128
                    tmp = sb.tile([128, 8, nt], f16, tag="tmp", name="tmp")
                    for m in range(8):
                        nc.vector.tensor_tensor(
                            tmp[:, m, :], w3win[:, m, e0:e0 + nt],
                            g_t[:, 0, e0:e0 + nt], OP.mult)
                    pmsgT = ps.tile([32, nt], f32, tag="pbig", name="pmsgT")
                    for m in range(8):
                        nc.tensor.matmul(pmsgT[:], masks_s[:, m * 32:(m + 1) * 32],
                                         tmp[:, m, :], start=(m == 0), stop=(m == 7))
                    msgT = sb.tile([32, nt], f32, tag="msgT", name="msgT")
                    nc.scalar.copy(msgT[:], pmsgT[:])
                    # batched transposes into one PSUM tile, single copy out
                    ptr = ps.tile([128, nst * 32], f32, tag="ptr", name="ptr")
                    for s in range(nst):
                        ncols = min(128, nt - s * 128)
                        nc.tensor.transpose(
                            ptr[0:ncols, s * 32:(s + 1) * 32],
                            msgT[:, s * 128:s * 128 + ncols], id32_s[:])
                    msg4 = sb.tile([128, nst * 32], f16, tag="msg4", name="msg4")
                    nc.scalar.copy(msg4[:], ptr[:])
                    for s in range(nst):
                        gs = (e_base + e0) // 128 + s
                        if st_resident:
                            st_sl = st_res[:, gs, :]
                        else:
                            st_t = sb.tile([128, 128], f16, tag="st", name="st_t")
                            nc.vector.tensor_scalar(
                                st_t[:], iota_s[:], dstl_s[:, gs:gs + 1],
                                invd_s[:, gs:gs + 1], op0=OP.is_equal,
                                op1=OP.mult)
                            st_sl = st_t[:]
                        nc.tensor.matmul(pwin[:], st_sl,
                                         msg4[:, s * 32:(s + 1) * 32],
                                         start=first, stop=False)
                        first = False
                # window tail: + h @ root_w + b (via resident augmented hT)
                if hT_resident:
                    htaug_sl = hT_res[:, w * 128:(w + 1) * 128]
                else:
                    hw_t = sb.tile([128, 32], f16, tag="hw", name="hw_t")
                    nc.sync.dma_start(
                        hw_t[:], h4own[d][w * 128:(w + 1) * 128, 0:32])
                    pthw = ps.tile([32, 128], f16, tag="ptp", name="pthw")
                    nc.tensor.transpose(pthw[:], hw_t[:], id128_s[:])
                    htaug_t = sb.tile([33, 128], f32, tag="htaug", name="htaug_t")
                    nc.scalar.copy(htaug_t[0:32, :], pthw[:])
                    nc.gpsimd.memset(htaug_t[32:33, :], 1.0)
                    htaug_sl = htaug_t[:]
                nc.tensor.matmul(pwin[:], htaug_sl,
                                 raug_s[:], start=False, stop=True)
                hnew = sb.tile([128, 128], f16, tag="hnew", name="hnew")
                nc.scalar.activation(hnew[:, 0:32], pwin[:], AF.Relu)
                if d < DEPTH - 1:
                    for r in range(1, 4):
                        nc.vector.tensor_copy(hnew[:, 32 * r:32 * (r + 1)],
                                              hnew[:, 0:32])
                    nc.sync.dma_start(
                        h4own[d + 1][w * 128:(w + 1) * 128, :], hnew[:])
                    if hT_resident:
                        # update resident hT for the next depth's root term
                        pth2 = ps.tile([32, 128], f16, tag="ptp", name="pth2")
                        nc.tensor.transpose(pth2[:], hnew[:, 0:32], id128_s[:])
                        nc.scalar.copy(hT_res[0:32, w * 128:(w + 1) * 128],
                                       pth2[:])
                else:
                    # final depth: fuse fc2 through a fresh transposed tile
                    pth2 = ps.tile([32, 128], f16, tag="ptp", name="pth2")
                    nc.tensor.transpose(pth2[:], hnew[:, 0:32], id128_s[:])
                    htaug = sb.tile([33, 128], f32, tag="htaug", name="htaug")
                    nc.scalar.copy(htaug[0:32, :], pth2[:])
                    nc.gpsimd.memset(htaug[32:33, :], 1.0)
                    py = ps.tile([128, 1], f32, tag="ptp", name="py")
                    nc.tensor.matmul(py[:], htaug[:], f2_s[:],
                                     start=True, stop=True)
                    y_sb = sb.tile([128, 1], f32, tag="ysb", name="y_sb")
                    nc.scalar.copy(y_sb[:], py[:])
                    nc.sync.dma_start(t_y.ap()[w * 128:(w + 1) * 128, :], y_sb[:])

            # ========== phase A fused with depth 0, per window ==========
            for w in range(nw):
                w3win = sb.tile([128, 8, EW[w]], f16, tag="w3win", name="w3win")
                for (e0, nt) in inner_tiles(w):
                    ea_t = sb.tile([KER_IN, nt], f16, tag="ea", name="ea_t")
                    nc.sync.dma_start(ea_t[:],
                                      t_eaT.ap()[:, int(ecum[w]) + e0:
                                                 int(ecum[w]) + e0 + nt])
                    h1_t = sb.tile([128, 2, nt], f16, tag="h1", name="h1_t")
                    for mo in range(2):
                        p1 = ps.tile([128, nt], f32, tag="pbig", name="p1")
                        nc.tensor.matmul(p1[:], k1w_s[:, mo * 128:(mo + 1) * 128],
                                         ea_t[:], start=True, stop=True)
                        nc.scalar.activation(h1_t[:, mo, :], p1[:], AF.Relu,
                                             bias=k1b_s[:, mo:mo + 1])
                    h2_t = sb.tile([128, 2, nt], f16, tag="h2", name="h2_t")
                    for mo in range(2):
                        p2 = ps.tile([128, nt], f32, tag="pbig", name="p2")
                        for mi in range(2):
                            nc.tensor.matmul(p2[:],
                                             k2w_s[:, mi, mo * 128:(mo + 1) * 128],
                                             h1_t[:, mi, :],
                                             start=(mi == 0), stop=(mi == 1))
                        nc.scalar.activation(h2_t[:, mo, :], p2[:], AF.Relu,
                                             bias=k2b_s[:, mo:mo + 1])
                    for mo in range(8):
                        p3 = ps.tile([128, nt], f32, tag="pbig", name="p3")
                        for mi in range(2):
                            nc.tensor.matmul(p3[:],
                                             k3w_s[:, mi, mo * 128:(mo + 1) * 128],
                                             h2_t[:, mi, :],
                                             start=(mi == 0), stop=(mi == 1))
                        out_sl = w3win[:, mo, e0:e0 + nt]
                        nc.scalar.activation(out_sl, p3[:], AF.Identity,
                                             bias=k3b_s[:, mo:mo + 1],
                                             scale=float(W3_SCALE))
                # store for depths 1-3 (fp8 cast; contiguous per partition)
                if w3_fp8:
                    nc.gpsimd.dma_start(w3w_dram[w][:, :, :], w3win[:])
                else:
                    nc.sync.dma_start(w3w_dram[w][:, :, :], w3win[:])
                # fused depth 0 for this window
                depth_window(0, w, w3win)
            if not prof:
                nc.gpsimd.collective_compute(
                    "AllGather", mybir.AluOpType.bypass, replica_groups=rg,
                    ins=[h4own[1].opt()], outs=[h4full[1].opt()])

            # ================= depths 1..3 =================
            for d in range(1, DEPTH):
                for w in range(nw):
                    w3win = sb.tile([128, 8, EW[w]], f16, tag="w3win",
                                    name="w3win_d")
                    if w3_fp8:
                        nc.gpsimd.dma_start(w3win[:], w3w_dram[w][:, :, :])
                    else:
                        nc.sync.dma_start(w3win[:], w3w_dram[w][:, :, :])
                    depth_window(d, w, w3win)
                if d < DEPTH - 1 and not prof:
                    nc.gpsimd.collective_compute(
                        "AllGather", mybir.AluOpType.bypass, replica_groups=rg,
                        ins=[h4own[d + 1].opt()], outs=[h4full[d + 1].opt()])

    nc.compile()
    return nc


_CACHE = {}


def _get_program(cfg, reps=1):
    key = (cfg["e_pc"], tuple(cfg["EW"]), cfg["n_cores"], cfg["npc"], reps,
           cfg.get("profile_single", False), cfg.get("w3_fp8", True),
           cfg.get("st_resident", True), cfg.get("hT_resident", True))
    if key not in _CACHE:
        _CACHE[key] = build_program(cfg, reps=reps)
    return _CACHE[key]


# ---------------------------------------------------------------------------
# Cached execution path. Under axon, run_bass_kernel_spmd rebuilds its jit
# wrapper on every call; this is the same bass2jax lowering with the jit
# built once and reused.
# ---------------------------------------------------------------------------
_RUNNERS = {}


def _make_runner(nc, n_cores):
    import jax
    import inspect
    from jax.sharding import Mesh, PartitionSpec, NamedSharding
    try:
        from jax import shard_map
    except ImportError:
        from jax.experimental.shard_map import shard_map
    _smkw = {}
    _params = inspect.signature(shard_map).parameters
    if "check_vma" in _params:
        _smkw["check_vma"] = False
    else:
        _smkw["check_rep"] = False
    from concourse import mybir
    from concourse.bass2jax import (_bass_exec_p, install_neuronx_cc_hook,
                                    partition_id_tensor)
    install_neuronx_cc_hook()

    partition_name = nc.partition_id_tensor.name if nc.partition_id_tensor else None
    in_names, out_names, out_avals = [], [], []
    for alloc in nc.m.functions[0].allocations:
        if not isinstance(alloc, mybir.MemoryLocationSet):
            continue
        name = alloc.memorylocations[0].name
        if alloc.kind == "ExternalInput":
            if name != partition_name:
                in_names.append(name)
        elif alloc.kind == "ExternalOutput":
            out_names.append(name)
            out_avals.append(jax.core.ShapedArray(tuple(alloc.tensor_shape),
                                                  mybir.dt.np(alloc.dtype)))
    n_params = len(in_names)
    n_outs = len(out_names)
    in_names_all = in_names + out_names
    if partition_name is not None:
        in_names_all.append(partition_name)

    def _body(*args):
        operands = list(args)
        if partition_name is not None:
            operands.append(partition_id_tensor())
        return tuple(_bass_exec_p.bind(
            *operands, out_avals=tuple(out_avals), in_names=tuple(in_names_all),
            out_names=tuple(out_names), lowering_input_output_aliases=(),
            sim_require_finite=True, sim_require_nnan=True, nc=nc))

    devices = jax.devices()[:n_cores]
    mesh = Mesh(np.asarray(devices), ("core",))
    donate = tuple(range(n_params, n_params + n_outs))
    sharded = jax.jit(
        shard_map(_body, mesh=mesh,
                  in_specs=(PartitionSpec("core"),) * (n_params + n_outs),
                  out_specs=(PartitionSpec("core"),) * n_outs,
                  **_smkw),
        donate_argnums=donate, keep_unused=True)

    zero_shapes = [(n_cores * a.shape[0], *a.shape[1:]) for a in out_avals]
    zero_dts = [a.dtype for a in out_avals]
    out_shapes = [tuple(a.shape) for a in out_avals]
    shard = NamedSharding(mesh, PartitionSpec("core"))

    class Runner:
        def __init__(self):
            self.sharded = sharded
            self.in_names = in_names
            self.out_names = out_names
            self.shard = shard

        def concat_inputs(self, in_maps):
            per_core = [[np.asarray(m[name]) for name in in_names]
                        for m in in_maps]
            return [np.concatenate([per_core[c][i] for c in range(n_cores)],
                                   axis=0) for i in range(n_params)]

        def zeros(self):
            return [np.zeros(s, d) for s, d in zip(zero_shapes, zero_dts)]

        def __call__(self, concat_in):
            out = sharded(*concat_in, *self.zeros())
            return [{name: np.asarray(out[i]).reshape(n_cores, *out_shapes[i])[c]
                     for i, name in enumerate(out_names)}
                    for c in range(n_cores)]

    return Runner()


def _get_runner(nc, n_cores):
    key = id(nc)
    if key not in _RUNNERS:
        _RUNNERS[key] = _make_runner(nc, n_cores)
    return _RUNNERS[key]


def kernel(**inputs):
    cfg, in_maps = host_prep(**inputs)
    nc = _get_program(cfg)
    try:
        run = _get_runner(nc, cfg["n_cores"])
        results = run(run.concat_inputs(in_maps))
    except Exception:
        import traceback
        traceback.print_exc()
        from concourse import bass_utils
        res = bass_utils.run_bass_kernel_spmd(
            nc, in_maps, core_ids=list(range(cfg["n_cores"])))
        results = res.results
    npc, n_cores = cfg["npc"], cfg["n_cores"]
    y = np.zeros((N, 1), np.float32)
    for k in range(n_cores):
        lo = k * npc
        hi = min(lo + npc, N)
        if hi > lo:
            y[lo:hi, 0] = results[k]["y"][:hi - lo, 0]
    return y
